# revision 2
# baseline (speedup 1.0000x reference)
"""GCN (6-layer GCNConv) Trainium2 Bass kernel — v5.

Data-parallel over batch (1 mesh per NeuronCore). Per layer
out = A_hat @ (x @ W) + b with A_hat = D^-1/2 (A+I) D^-1/2 shared across batch
and layers.

v4 structure (per core), informed by v2/v3 traces:
  - All wide matmuls in bf16 (fp32 is 4 cyc/row on the PE; bf16 is 1).
  - Per (dst-tile, layer) the message gather is ONE indirect_dma_start with a
    [128, C] offset AP (the DGE CounterMachine emits descriptors at ~0.34ns
    each vs ~9.5ns for the DMAGatherAnt ucode loop; one call per tile leaves
    only the ~1us fixed SWDGE cost). out[p, c*F:(c+1)*F] = h[gsrc[p, c]].
  - Self-loops live in the edge list (C 6->7 costs the same slots as
    6+separate-loop) so v2's per-tile hre read + dinv2 scaling is gone.
  - One-hot built per chunk with a two-op tensor_scalar
    (iota is_eq slot) * norm from per-partition scalar columns — step-1
    16-bit single-src => DVE 4x mode, vs v2/v3's broadcast tensor_tensor
    pinned at 1x.
  - Bias enters the scatter PSUM group as a ones-row matmul (lhsT=[1,128]
    ones, rhs=[1,F] bias row), so the PSUM->SBUF node copy + ReLU fuse into
    one scalar-engine activation.
  - Layers 5/6 (64-wide) keep fp32 h5/x6 tables in DRAM; messages are cast
    to bf16 on-chip for the scatter matmuls.
"""
import os
import sys
import time

sys.path.insert(0, "/opt/trn_rl_repo")
import numpy as np
from contextlib import ExitStack

import concourse.bass as bass
import concourse.mybir as mybir
import concourse.tile as tile
from concourse.bass_utils import run_bass_kernel_spmd
from concourse.library_config import mlp as _mlp_lib

P = 128
F32 = mybir.dt.float32
BF16 = mybir.dt.bfloat16
I32 = mybir.dt.int32
I16 = mybir.dt.int16

_msw_ctr = [0]


def _split_multiwaits(nc, max_waits=1):
    """This walrus build rejects >1 sync wait per instruction: split extras
    onto preceding same-engine NOPs."""
    for f in nc.m.functions:
        for b in f.blocks:
            out, changed = [], False
            for inst in b.instructions:
                si = getattr(inst, "sync_info", None)
                waits = list(si.on_wait) if si is not None else []
                if len(waits) > max_waits:
                    changed = True
                    for w in waits[:-max_waits]:
                        _msw_ctr[0] += 1
                        nop = mybir.InstNoOp(name=f"msw-{_msw_ctr[0]}", ins=[], outs=[])
                        nop.engine = inst.engine
                        nop.sync_info = mybir.SyncInfo(on_wait=[w], on_update=[])
                        out.append(nop)
                    si.on_wait = waits[-max_waits:]
                out.append(inst)
            if changed:
                b.instructions = out
    return nc


def _pack_graph(src, dst, N):
    """Relabel nodes into degree-balanced 128-node tiles. Self-loops are part
    of the edge list. Edges are grouped per dst tile, padded to C chunks of
    128 (dummies: src 0, norm 0)."""
    T = (N + P - 1) // P
    NP = T * P
    indeg = np.bincount(dst, minlength=N)          # real in-edges only
    C = max(1, int(np.ceil(len(src) / (T * P))))

    order = np.argsort(-indeg, kind="stable")
    while True:
        cap = C * P
        load = np.zeros(T, np.int64)
        count = np.zeros(T, np.int64)
        assign = np.empty(N, np.int64)
        ok = True
        for v in order:
            d = int(indeg[v])
            best_t, best_rem = -1, -1
            for t in range(T):
                if count[t] < P:
                    rem = cap - load[t]
                    if rem > best_rem:
                        best_rem, best_t = rem, t
            if best_t < 0 or load[best_t] + d > cap:
                ok = False
                break
            assign[v] = best_t
            load[best_t] += d
            count[best_t] += 1
        if ok:
            break
        C += 1

    perm = np.full(NP, -1, np.int64)
    new_of_old = np.empty(N, np.int64)
    cursor = np.zeros(T, np.int64)
    for v in range(N):
        t = assign[v]
        nid = t * P + cursor[t]
        cursor[t] += 1
        perm[nid] = v
        new_of_old[v] = nid

    deg = (indeg + 1).astype(np.float32)           # GCN degree includes self-loop
    dinv = (1.0 / np.sqrt(deg, dtype=np.float32)).astype(np.float32)
    norm = (dinv[src] * dinv[dst]).astype(np.float32)

    src_n = new_of_old[src]
    dst_n = new_of_old[dst]
    tile_of_e = dst_n // P
    order_e = np.argsort(tile_of_e, kind="stable")
    src_n, dst_n, norm = src_n[order_e], dst_n[order_e], norm[order_e]
    tile_of_e = tile_of_e[order_e]

    gsrc = np.zeros((T, C, P), np.int32)
    slot = np.zeros((T, C, P), np.float32)
    nrm = np.zeros((T, C, P), np.float32)
    starts = np.searchsorted(tile_of_e, np.arange(T + 1))
    for t in range(T):
        lo, hi = starts[t], starts[t + 1]
        n_e = hi - lo
        assert n_e <= C * P, (t, n_e, C * P)
        fs = np.zeros(C * P, np.int32)
        fl = np.zeros(C * P, np.float32)
        fn = np.zeros(C * P, np.float32)
        fs[:n_e] = src_n[lo:hi]
        fl[:n_e] = (dst_n[lo:hi] - t * P).astype(np.float32)
        fn[:n_e] = norm[lo:hi]
        gsrc[t] = fs.reshape(C, P)
        slot[t] = fl.reshape(C, P)
        nrm[t] = fn.reshape(C, P)

    # per-(slot, tile) 1/deg for the on-device self-loop diagonal (0 for dummies)
    dinv_new = np.zeros(NP, np.float32)
    valid = perm >= 0
    dinv_new[valid] = dinv[perm[valid]]
    dinv2 = (dinv_new ** 2).reshape(T, P).T.copy()   # [128, T]

    assert NP - 1 <= np.iinfo(np.int16).max
    IW = C * P // 16
    gidx = np.zeros((16, T * IW), np.int16)
    for t in range(T):
        lin = gsrc[t].reshape(C * P)                # lin[c*128+p]
        gidx[:, t * IW:(t + 1) * IW] = lin.reshape(IW, 16).T
    gidx = np.ascontiguousarray(np.tile(gidx, (8, 1)))

    def dev(a):
        return np.ascontiguousarray(a.transpose(2, 0, 1).reshape(P, -1))

    return dict(NP=NP, T=T, C=C, perm=perm, dinv2=np.ascontiguousarray(dinv2),
                gidx=gidx, slot=dev(slot), norm=dev(nrm))


def _build_nc(NP, T, C, FM, F5, FO):
    scratch = int(os.environ.get("KBASS_SCRATCH", "16384"))
    SBUFS = int(os.environ.get("KBASS_BUFS", "5"))
    PSBUFS = int(os.environ.get("KBASS_PSBUFS", "3"))
    HASB = os.environ.get("KBASS_HASBIAS", "0") == "1"
    nc = bass.Bass(dynamic_dma_scratch_size=scratch)
    TC = T * C
    KM = FM // P

    d = {}
    d["xT1"] = nc.dram_tensor("xT1", [3, NP], BF16, kind="ExternalInput")
    d["hcrep"] = nc.dram_tensor("hcrep", [P, FM], F32, kind="ExternalInput")
    d["W1v"] = nc.dram_tensor("W1v", [3, FM], BF16, kind="ExternalInput")
    for i in (2, 3, 4):
        d[f"W{i}"] = nc.dram_tensor(f"W{i}", [FM, FM], BF16, kind="ExternalInput")
    d["W5"] = nc.dram_tensor("W5", [FM, F5], BF16, kind="ExternalInput")
    d["W6"] = nc.dram_tensor("W6", [F5, FO], F32, kind="ExternalInput")
    d["b6rep"] = nc.dram_tensor("b6rep", [P, FO], F32, kind="ExternalInput")
    IW = C * P // 16
    d["gidx"] = nc.dram_tensor("gidx", [P, T * IW], I16, kind="ExternalInput")
    d["slotv"] = nc.dram_tensor("slotv", [P, TC], F32, kind="ExternalInput")
    d["normv"] = nc.dram_tensor("normv", [P, TC], F32, kind="ExternalInput")
    d["iotab"] = nc.dram_tensor("iotab", [P, P], BF16, kind="ExternalInput")
    d["identb"] = nc.dram_tensor("identb", [P, P], BF16, kind="ExternalInput")
    d["onesb"] = nc.dram_tensor("onesb", [1, P], BF16, kind="ExternalInput")
    # bias rows (bf16): [1, 4*FM] for layers 1-4, [1, F5] for layer 5
    d["brow14"] = nc.dram_tensor("brow14", [1, 4 * FM], BF16, kind="ExternalInput")
    d["brow5"] = nc.dram_tensor("brow5", [1, F5], BF16, kind="ExternalInput")
    d["dinv2"] = nc.dram_tensor("dinv2", [P, T], F32, kind="ExternalInput")
    d["pcol"] = nc.dram_tensor("pcol", [P, 1], F32, kind="ExternalInput")
    out_d = nc.dram_tensor("out", [NP, FO], F32, kind="ExternalOutput")

    h512 = [nc.dram_tensor(f"h{i}", [NP, FM], BF16, kind="Internal")
            for i in (1, 2, 3, 4)]
    h5_d = nc.dram_tensor("h5", [NP, 2 * F5], BF16, kind="Internal")
    x6_d = nc.dram_tensor("x6", [NP, 2 * F5], BF16, kind="Internal")

    Ident = mybir.ActivationFunctionType.Identity
    Relu = mybir.ActivationFunctionType.Relu

    with tile.TileContext(nc) as tc:
        nc.gpsimd.load_library(_mlp_lib)
        with ExitStack() as ctx:
            res = ctx.enter_context(tc.tile_pool(name="res", bufs=1))
            gidx_sb = res.tile([P, T * IW], I16)
            slot_sb = res.tile([P, TC], F32)
            norm_sb = res.tile([P, TC], F32)
            iota_sb = res.tile([P, P], BF16)
            ident_sb = res.tile([P, P], BF16)
            ones_sb = res.tile([1, P], BF16)
            brow14_sb = res.tile([1, 4 * FM], BF16)
            brow5_sb = res.tile([1, F5], BF16)
            hcrep_sb = res.tile([P, FM], F32)
            b6rep_sb = res.tile([P, FO], F32)
            dinv2_sb = res.tile([P, T], F32)
            pcol_sb = res.tile([P, 1], F32)
            for name, t_sb in [("gidx", gidx_sb), ("slotv", slot_sb),
                               ("normv", norm_sb), ("iotab", iota_sb),
                               ("identb", ident_sb), ("onesb", ones_sb),
                               ("brow14", brow14_sb), ("brow5", brow5_sb),
                               ("hcrep", hcrep_sb), ("b6rep", b6rep_sb),
                               ("dinv2", dinv2_sb), ("pcol", pcol_sb)]:
                nc.sync.dma_start(out=t_sb[:], in_=d[name][:, :])

            nidx_reg = nc.gpsimd.to_reg(C * P)

            def gather(sp, t, src_d, felem, dt, tag):
                """msg[p, c*felem:(c+1)*felem] = src[idx[c*128+p], :]"""
                m = sp.tile([P, C * felem], dt, tag=tag, name=f"{tag}_{t}")
                nc.gpsimd.dma_gather(
                    out_ap=m[:].rearrange("p (c f) -> p c f", c=C),
                    in_ap=src_d[:, :],
                    idxs_ap=gidx_sb[:, t * IW:(t + 1) * IW],
                    num_idxs=C * P,
                    num_idxs_reg=nidx_reg,
                    elem_size=felem,
                    single_packet=False,
                )
                return m

            def build_diag(sp, t, tag):
                """diag[p, j] = (j==p) / deg[t*128+p], bf16."""
                dg = sp.tile([P, P], BF16, tag=tag, name=f"{tag}_{t}")
                nc.vector.tensor_scalar(
                    out=dg[:],
                    in0=iota_sb[:],
                    scalar1=pcol_sb[:, 0:1],
                    scalar2=dinv2_sb[:, t:t + 1],
                    op0=mybir.AluOpType.is_equal,
                    op1=mybir.AluOpType.mult,
                )
                return dg

            def build_onehot(sp, t, tag):
                """oh[p, c*P+j] = (slot[p,c]==j) * norm[p,c], bf16."""
                oh = sp.tile([P, C * P], BF16, tag=tag, name=f"{tag}_{t}")
                for c in range(C):
                    nc.vector.tensor_scalar(
                        out=oh[:, c * P:(c + 1) * P],
                        in0=iota_sb[:],
                        scalar1=slot_sb[:, t * C + c:t * C + c + 1],
                        scalar2=norm_sb[:, t * C + c:t * C + c + 1],
                        op0=mybir.AluOpType.is_equal,
                        op1=mybir.AluOpType.mult,
                    )
                return oh

            # ---- layer 1 dense: h1 = verts @ W1[:3] + (img @ W1[3:]) ----
            with tc.tile_pool(name="l1", bufs=1) as l1p, \
                 tc.tile_pool(name="l1ps", bufs=2, space="PSUM") as l1ps, \
                 tc.tile_pool(name="l1sb", bufs=3) as l1sb:
                xT1_sb = l1p.tile([3, NP], BF16)
                nc.sync.dma_start(out=xT1_sb[:], in_=d["xT1"][:, :])
                W1v_sb = l1p.tile([3, FM], BF16)
                nc.sync.dma_start(out=W1v_sb[:], in_=d["W1v"][:, :])
                for n in range(T):
                    ph = l1ps.tile([P, FM], F32, tag="ph")
                    nc.tensor.matmul(out=ph[:], lhsT=xT1_sb[:, n * P:(n + 1) * P],
                                     rhs=W1v_sb[:], start=True, stop=True)
                    hs = l1sb.tile([P, FM], BF16, tag="hs")
                    nc.vector.tensor_add(out=hs[:], in0=ph[:], in1=hcrep_sb[:])
                    nc.sync.dma_start(out=h512[0][n * P:(n + 1) * P, :], in_=hs[:])

            # ---- phases i=1..4: scatter(i) + dense(i+1) per dst tile ----
            for i in (1, 2, 3, 4):
                relu = i in (2, 4)
                h_src = h512[i - 1]
                F_out = FM if i < 4 else F5
                h_dst = h512[i] if i < 4 else h5_d
                W_d = d[f"W{i + 1}"]
                with tc.tile_pool(name=f"ph{i}", bufs=SBUFS) as sp, \
                     tc.tile_pool(name=f"ph{i}w", bufs=1) as wp, \
                     tc.tile_pool(name=f"ph{i}ps", bufs=PSBUFS, space="PSUM") as pp, \
                     tc.tile_pool(name=f"ph{i}pt", bufs=2, space="PSUM") as pt, \
                     tc.tile_pool(name=f"ph{i}pd", bufs=2, space="PSUM") as pd:
                    W_sb = [wp.tile([P, F_out], BF16, tag=f"w{k}", name=f"w{i}_{k}")
                            for k in range(KM)]
                    for k in range(KM):
                        nc.sync.dma_start(out=W_sb[k][:], in_=W_d[k * P:(k + 1) * P, :])
                    for t in range(T):
                        msg = gather(sp, t, h_src, FM, BF16, "msg")
                        hre = sp.tile([P, FM], BF16, tag="hre", name=f"hre{i}_{t}")
                        nc.sync.dma_start(out=hre[:],
                                          in_=h_src[t * P:(t + 1) * P, :])
                        oh = build_onehot(sp, t, "oh")
                        dg = build_diag(sp, t, "dg")
                        pa = pp.tile([P, FM], F32, tag="pa", name=f"pa{i}_{t}")
                        if HASB:
                            nc.tensor.matmul(
                                out=pa[:], lhsT=ones_sb[:],
                                rhs=brow14_sb[:, (i - 1) * FM:i * FM],
                                start=True, stop=False)
                        nc.tensor.matmul(
                            out=pa[:], lhsT=dg[:], rhs=hre[:],
                            start=not HASB, stop=False)
                        for c in range(C):
                            nc.tensor.matmul(
                                out=pa[:], lhsT=oh[:, c * P:(c + 1) * P],
                                rhs=msg[:, c * FM:(c + 1) * FM],
                                start=False, stop=(c == C - 1))
                        node = sp.tile([P, FM], BF16, tag="node", name=f"nd{i}_{t}")
                        nc.scalar.activation(out=node[:], in_=pa[:],
                                             func=Relu if relu else Ident, bias=0.0)
                        ptr = pt.tile([P, FM], BF16, tag="ptr", name=f"pt{i}_{t}")
                        stage = sp.tile([P, FM], BF16, tag="stage", name=f"st{i}_{t}")
                        for fo in range(KM):
                            nc.tensor.matmul(
                                out=ptr[:, fo * P:(fo + 1) * P],
                                lhsT=node[:, fo * P:(fo + 1) * P],
                                rhs=ident_sb[:], is_transpose=True,
                                start=True, stop=True)
                            nc.scalar.activation(
                                out=stage[:, fo * P:(fo + 1) * P],
                                in_=ptr[:, fo * P:(fo + 1) * P],
                                func=Ident, bias=0.0)
                        ph = pd.tile([P, F_out], F32, tag="ph", name=f"pd{i}_{t}")
                        for k in range(KM):
                            nc.tensor.matmul(out=ph[:], lhsT=stage[:, k * P:(k + 1) * P],
                                             rhs=W_sb[k][:], start=(k == 0),
                                             stop=(k == KM - 1))
                        if i < 4:
                            hs = sp.tile([P, F_out], BF16, tag="hs",
                                         name=f"hs{i}_{t}")
                            nc.vector.tensor_copy(out=hs[:], in_=ph[:])
                            nc.sync.dma_start(out=h_dst[t * P:(t + 1) * P, :],
                                              in_=hs[:])
                        else:
                            hs = sp.tile([P, 2 * F_out], BF16, tag="hs",
                                         name=f"hs{i}_{t}")
                            nc.vector.tensor_copy(out=hs[:, :F_out], in_=ph[:])
                            nc.vector.memset(hs[:, F_out:], 0.0)
                            nc.sync.dma_start(out=h_dst[t * P:(t + 1) * P, :],
                                              in_=hs[:])

            # ---- layer 5 scatter: x6 = A @ h5 + b5 ----
            with tc.tile_pool(name="s5", bufs=SBUFS) as sp5, \
                 tc.tile_pool(name="s5ps", bufs=PSBUFS, space="PSUM") as pp5:
                for t in range(T):
                    msg5b = gather(sp5, t, h5_d, 2 * F5, BF16, "m5")
                    hre5 = sp5.tile([P, F5], BF16, tag="hre5", name=f"hr5_{t}")
                    nc.sync.dma_start(out=hre5[:],
                                      in_=h5_d[t * P:(t + 1) * P, :F5])
                    oh5 = build_onehot(sp5, t, "oh5")
                    dg5 = build_diag(sp5, t, "dg5")
                    pa5 = pp5.tile([P, F5], F32, tag="pa5", name=f"pa5_{t}")
                    if HASB:
                        nc.tensor.matmul(out=pa5[:], lhsT=ones_sb[:], rhs=brow5_sb[:],
                                         start=True, stop=False)
                    nc.tensor.matmul(out=pa5[:], lhsT=dg5[:], rhs=hre5[:],
                                     start=not HASB, stop=False)
                    for c in range(C):
                        nc.tensor.matmul(out=pa5[:], lhsT=oh5[:, c * P:(c + 1) * P],
                                         rhs=msg5b[:, 2 * c * F5:2 * c * F5 + F5],
                                         start=False, stop=(c == C - 1))
                    x6t = sp5.tile([P, 2 * F5], BF16, tag="x6t", name=f"x6_{t}")
                    nc.scalar.activation(out=x6t[:, :F5], in_=pa5[:], func=Ident,
                                         bias=0.0)
                    nc.vector.memset(x6t[:, F5:], 0.0)
                    nc.sync.dma_start(out=x6_d[t * P:(t + 1) * P, :], in_=x6t[:])

            # ---- layer 6: feature-major scatter, then 64->3 dense ----
            with tc.tile_pool(name="s6", bufs=SBUFS) as sp6, \
                 tc.tile_pool(name="s6w", bufs=1) as wp6, \
                 tc.tile_pool(name="s6ps", bufs=2, space="PSUM") as pp6, \
                 tc.tile_pool(name="s6pd", bufs=2, space="PSUM") as pd6:
                W6_sb = wp6.tile([F5, FO], F32)
                nc.sync.dma_start(out=W6_sb[:], in_=d["W6"][:, :])
                for t in range(T):
                    msg6b = gather(sp6, t, x6_d, 2 * F5, BF16, "m6")
                    hre6 = sp6.tile([P, F5], BF16, tag="hre6", name=f"hr6_{t}")
                    nc.sync.dma_start(out=hre6[:],
                                      in_=x6_d[t * P:(t + 1) * P, :F5])
                    oh6 = build_onehot(sp6, t, "oh6")
                    dg6 = build_diag(sp6, t, "dg6")
                    pg = pp6.tile([F5, P], F32, tag="pg", name=f"pg_{t}")
                    nc.tensor.matmul(out=pg[:], lhsT=hre6[:], rhs=dg6[:],
                                     start=True, stop=False)
                    for c in range(C):
                        nc.tensor.matmul(out=pg[:], lhsT=msg6b[:, 2 * c * F5:2 * c * F5 + F5],
                                         rhs=oh6[:, c * P:(c + 1) * P],
                                         start=False, stop=(c == C - 1))
                    gst = sp6.tile([F5, P], F32, tag="gst", name=f"g_{t}")
                    nc.scalar.activation(out=gst[:], in_=pg[:], func=Ident, bias=0.0)
                    pf = pd6.tile([P, FO], F32, tag="pf", name=f"pf_{t}")
                    nc.tensor.matmul(out=pf[:], lhsT=gst[:], rhs=W6_sb[:],
                                     start=True, stop=True)
                    os_ = sp6.tile([P, FO], F32, tag="os", name=f"o_{t}")
                    nc.vector.tensor_add(out=os_[:], in0=pf[:], in1=b6rep_sb[:])
                    nc.sync.dma_start(out=out_d[t * P:(t + 1) * P, :], in_=os_[:])

    mybir.codegen_inst_isa_subclasses(nc)
    _split_multiwaits(nc)
    return nc


def _bf16(a):
    import ml_dtypes
    return np.ascontiguousarray(np.asarray(a, np.float32).astype(ml_dtypes.bfloat16))


def _prepare(batch_vertices, img_features, edge_indices,
             W1, b1, W2, b2, W3, b3, W4, b4, W5, b5, W6, b6):
    B, N, _ = batch_vertices.shape
    FM = W1.shape[1]
    F5 = W5.shape[1]
    FO = W6.shape[1]

    ei = np.asarray(edge_indices).astype(np.int64)
    g = _pack_graph(ei[0], ei[1], N)
    NP, T, C, perm = g["NP"], g["T"], g["C"], g["perm"]

    hc = img_features.astype(np.float32) @ W1[3:].astype(np.float32)

    valid = perm >= 0
    vperm = np.zeros((B, NP, 3), np.float32)
    vperm[:, valid, :] = batch_vertices[:, perm[valid], :]

    iota = np.tile(np.arange(P, dtype=np.float32), (P, 1))
    ident = np.eye(P, dtype=np.float32)

    common = {
        "W1v": _bf16(W1[:3]),
        "W2": _bf16(W2),
        "W3": _bf16(W3),
        "W4": _bf16(W4),
        "W5": _bf16(W5),
        "W6": np.ascontiguousarray(W6.astype(np.float32)),
        "b6rep": np.tile(b6.astype(np.float32), (P, 1)),
        "gidx": g["gidx"],
        "slotv": np.ascontiguousarray(g["slot"]),
        "normv": np.ascontiguousarray(g["norm"]),
        "iotab": _bf16(iota),
        "identb": _bf16(ident),
        "onesb": _bf16(np.ones((1, P), np.float32)),
        "brow14": _bf16(np.concatenate([b1, b2, b3, b4])[None, :]),
        "brow5": _bf16(b5[None, :]),
        "dinv2": g["dinv2"],
        "pcol": np.arange(P, dtype=np.float32)[:, None].copy(),
    }
    in_maps = []
    for b in range(B):
        m = dict(common)
        m["xT1"] = _bf16(vperm[b].T)
        m["hcrep"] = np.tile(hc[b], (P, 1)).astype(np.float32)
        in_maps.append(m)
    meta = dict(NP=NP, T=T, C=C, perm=perm, valid=valid, B=B, N=N,
                FM=FM, F5=F5, FO=FO)
    return in_maps, meta


_BUILD_CACHE = {}


def run(inputs, trace=False):
    in_maps, meta = _prepare(**inputs)
    hasb = any(np.any(np.asarray(inputs[f"b{i}"])) for i in range(1, 6))
    os.environ["KBASS_HASBIAS"] = "1" if hasb else "0"
    key = (meta["NP"], meta["C"], meta["FM"], meta["F5"], meta["FO"], hasb)
    if key not in _BUILD_CACHE:
        t0 = time.time()
        _BUILD_CACHE[key] = _build_nc(meta["NP"], meta["T"], meta["C"],
                                      meta["FM"], meta["F5"], meta["FO"])
        print(f"[kernel] built bass program in {time.time()-t0:.1f}s", file=sys.stderr)
    nc = _BUILD_CACHE[key]
    B = meta["B"]
    res = run_bass_kernel_spmd(nc, in_maps, core_ids=list(range(B)), trace=trace)
    perm, valid, N = meta["perm"], meta["valid"], meta["N"]
    out = np.empty((B, N, meta["FO"]), np.float32)
    for b in range(B):
        dev = res.results[b]["out"]
        out[b, perm[valid], :] = dev[valid, :]
    return out, res


def kernel(**inputs) -> np.ndarray:
    out, _ = run(inputs)
    return out


# revision 3
# speedup vs baseline: 1.0261x; 1.0261x over previous
"""GCN (6-layer GCNConv) Trainium2 Bass kernel — v5.

Data-parallel over batch (1 mesh per NeuronCore). Per layer
out = A_hat @ (x @ W) + b with A_hat = D^-1/2 (A+I) D^-1/2 shared across batch
and layers.

v4 structure (per core), informed by v2/v3 traces:
  - All wide matmuls in bf16 (fp32 is 4 cyc/row on the PE; bf16 is 1).
  - Per (dst-tile, layer) the message gather is ONE indirect_dma_start with a
    [128, C] offset AP (the DGE CounterMachine emits descriptors at ~0.34ns
    each vs ~9.5ns for the DMAGatherAnt ucode loop; one call per tile leaves
    only the ~1us fixed SWDGE cost). out[p, c*F:(c+1)*F] = h[gsrc[p, c]].
  - Self-loops live in the edge list (C 6->7 costs the same slots as
    6+separate-loop) so v2's per-tile hre read + dinv2 scaling is gone.
  - One-hot built per chunk with a two-op tensor_scalar
    (iota is_eq slot) * norm from per-partition scalar columns — step-1
    16-bit single-src => DVE 4x mode, vs v2/v3's broadcast tensor_tensor
    pinned at 1x.
  - Bias enters the scatter PSUM group as a ones-row matmul (lhsT=[1,128]
    ones, rhs=[1,F] bias row), so the PSUM->SBUF node copy + ReLU fuse into
    one scalar-engine activation.
  - Layers 5/6 (64-wide) keep fp32 h5/x6 tables in DRAM; messages are cast
    to bf16 on-chip for the scatter matmuls.
"""
import os
import sys
import time

sys.path.insert(0, "/opt/trn_rl_repo")
import numpy as np
from contextlib import ExitStack

import concourse.bass as bass
import concourse.mybir as mybir
import concourse.tile as tile
from concourse.bass_utils import run_bass_kernel_spmd
from concourse.library_config import mlp as _mlp_lib

P = 128
F32 = mybir.dt.float32
BF16 = mybir.dt.bfloat16
I32 = mybir.dt.int32
I16 = mybir.dt.int16

_msw_ctr = [0]


def _split_multiwaits(nc, max_waits=1):
    """This walrus build rejects >1 sync wait per instruction: split extras
    onto preceding same-engine NOPs."""
    for f in nc.m.functions:
        for b in f.blocks:
            out, changed = [], False
            for inst in b.instructions:
                si = getattr(inst, "sync_info", None)
                waits = list(si.on_wait) if si is not None else []
                if len(waits) > max_waits:
                    changed = True
                    for w in waits[:-max_waits]:
                        _msw_ctr[0] += 1
                        nop = mybir.InstNoOp(name=f"msw-{_msw_ctr[0]}", ins=[], outs=[])
                        nop.engine = inst.engine
                        nop.sync_info = mybir.SyncInfo(on_wait=[w], on_update=[])
                        out.append(nop)
                    si.on_wait = waits[-max_waits:]
                out.append(inst)
            if changed:
                b.instructions = out
    return nc


def _pack_graph(src, dst, N):
    """Relabel nodes into degree-balanced 128-node tiles. Self-loops are part
    of the edge list. Edges are grouped per dst tile, padded to C chunks of
    128 (dummies: src 0, norm 0)."""
    T = (N + P - 1) // P
    NP = T * P
    indeg = np.bincount(dst, minlength=N)          # real in-edges only
    C = max(1, int(np.ceil(len(src) / (T * P))))

    order = np.argsort(-indeg, kind="stable")
    while True:
        cap = C * P
        load = np.zeros(T, np.int64)
        count = np.zeros(T, np.int64)
        assign = np.empty(N, np.int64)
        ok = True
        for v in order:
            d = int(indeg[v])
            best_t, best_rem = -1, -1
            for t in range(T):
                if count[t] < P:
                    rem = cap - load[t]
                    if rem > best_rem:
                        best_rem, best_t = rem, t
            if best_t < 0 or load[best_t] + d > cap:
                ok = False
                break
            assign[v] = best_t
            load[best_t] += d
            count[best_t] += 1
        if ok:
            break
        C += 1

    perm = np.full(NP, -1, np.int64)
    new_of_old = np.empty(N, np.int64)
    cursor = np.zeros(T, np.int64)
    for v in range(N):
        t = assign[v]
        nid = t * P + cursor[t]
        cursor[t] += 1
        perm[nid] = v
        new_of_old[v] = nid

    deg = (indeg + 1).astype(np.float32)           # GCN degree includes self-loop
    dinv = (1.0 / np.sqrt(deg, dtype=np.float32)).astype(np.float32)
    norm = (dinv[src] * dinv[dst]).astype(np.float32)

    src_n = new_of_old[src]
    dst_n = new_of_old[dst]
    tile_of_e = dst_n // P
    order_e = np.argsort(tile_of_e, kind="stable")
    src_n, dst_n, norm = src_n[order_e], dst_n[order_e], norm[order_e]
    tile_of_e = tile_of_e[order_e]

    gsrc = np.zeros((T, C, P), np.int32)
    slot = np.zeros((T, C, P), np.float32)
    nrm = np.zeros((T, C, P), np.float32)
    starts = np.searchsorted(tile_of_e, np.arange(T + 1))
    for t in range(T):
        lo, hi = starts[t], starts[t + 1]
        n_e = hi - lo
        assert n_e <= C * P, (t, n_e, C * P)
        fs = np.zeros(C * P, np.int32)
        fl = np.zeros(C * P, np.float32)
        fn = np.zeros(C * P, np.float32)
        fs[:n_e] = src_n[lo:hi]
        fl[:n_e] = (dst_n[lo:hi] - t * P).astype(np.float32)
        fn[:n_e] = norm[lo:hi]
        gsrc[t] = fs.reshape(C, P)
        slot[t] = fl.reshape(C, P)
        nrm[t] = fn.reshape(C, P)

    # per-(slot, tile) 1/deg for the on-device self-loop diagonal (0 for dummies)
    dinv_new = np.zeros(NP, np.float32)
    valid = perm >= 0
    dinv_new[valid] = dinv[perm[valid]]
    dinv2 = (dinv_new ** 2).reshape(T, P).T.copy()   # [128, T]

    assert NP - 1 <= np.iinfo(np.int16).max
    IW = C * P // 16
    gidx = np.zeros((16, T * IW), np.int16)
    for t in range(T):
        lin = gsrc[t].reshape(C * P)                # lin[c*128+p]
        gidx[:, t * IW:(t + 1) * IW] = lin.reshape(IW, 16).T
    gidx = np.ascontiguousarray(np.tile(gidx, (8, 1)))

    def dev(a):
        return np.ascontiguousarray(a.transpose(2, 0, 1).reshape(P, -1))

    return dict(NP=NP, T=T, C=C, perm=perm, dinv2=np.ascontiguousarray(dinv2),
                gidx=gidx, slot=dev(slot), norm=dev(nrm))


def _build_nc(NP, T, C, FM, F5, FO):
    scratch = int(os.environ.get("KBASS_SCRATCH", "16384"))
    SBUFS = int(os.environ.get("KBASS_BUFS", "3"))
    PSBUFS = int(os.environ.get("KBASS_PSBUFS", "3"))
    HASB = os.environ.get("KBASS_HASBIAS", "0") == "1"
    nc = bass.Bass(dynamic_dma_scratch_size=scratch)
    TC = T * C
    KM = FM // P

    d = {}
    d["xT1"] = nc.dram_tensor("xT1", [3, NP], BF16, kind="ExternalInput")
    d["hcrep"] = nc.dram_tensor("hcrep", [P, FM], F32, kind="ExternalInput")
    d["W1v"] = nc.dram_tensor("W1v", [3, FM], BF16, kind="ExternalInput")
    for i in (2, 3, 4):
        d[f"W{i}"] = nc.dram_tensor(f"W{i}", [FM, FM], BF16, kind="ExternalInput")
    d["W5"] = nc.dram_tensor("W5", [FM, F5], BF16, kind="ExternalInput")
    d["W6"] = nc.dram_tensor("W6", [F5, FO], F32, kind="ExternalInput")
    d["b6rep"] = nc.dram_tensor("b6rep", [P, FO], F32, kind="ExternalInput")
    IW = C * P // 16
    d["gidx"] = nc.dram_tensor("gidx", [P, T * IW], I16, kind="ExternalInput")
    d["slotv"] = nc.dram_tensor("slotv", [P, TC], F32, kind="ExternalInput")
    d["normv"] = nc.dram_tensor("normv", [P, TC], F32, kind="ExternalInput")
    d["iotab"] = nc.dram_tensor("iotab", [P, P], BF16, kind="ExternalInput")
    d["identb"] = nc.dram_tensor("identb", [P, P], BF16, kind="ExternalInput")
    d["onesb"] = nc.dram_tensor("onesb", [1, P], BF16, kind="ExternalInput")
    # bias rows (bf16): [1, 4*FM] for layers 1-4, [1, F5] for layer 5
    d["brow14"] = nc.dram_tensor("brow14", [1, 4 * FM], BF16, kind="ExternalInput")
    d["brow5"] = nc.dram_tensor("brow5", [1, F5], BF16, kind="ExternalInput")
    d["dinv2"] = nc.dram_tensor("dinv2", [P, T], F32, kind="ExternalInput")
    d["pcol"] = nc.dram_tensor("pcol", [P, 1], F32, kind="ExternalInput")
    out_d = nc.dram_tensor("out", [NP, FO], F32, kind="ExternalOutput")

    h512 = [nc.dram_tensor(f"h{i}", [NP, FM], BF16, kind="Internal")
            for i in (1, 2, 3, 4)]
    h5_d = nc.dram_tensor("h5", [NP, 2 * F5], BF16, kind="Internal")
    x6_d = nc.dram_tensor("x6", [NP, 2 * F5], BF16, kind="Internal")

    Ident = mybir.ActivationFunctionType.Identity
    Relu = mybir.ActivationFunctionType.Relu

    with tile.TileContext(nc) as tc:
        nc.gpsimd.load_library(_mlp_lib)
        with ExitStack() as ctx:
            res = ctx.enter_context(tc.tile_pool(name="res", bufs=1))
            gidx_sb = res.tile([P, T * IW], I16)
            slot_sb = res.tile([P, TC], F32)
            norm_sb = res.tile([P, TC], F32)
            iota_sb = res.tile([P, P], BF16)
            ident_sb = res.tile([P, P], BF16)
            ones_sb = res.tile([1, P], BF16)
            brow14_sb = res.tile([1, 4 * FM], BF16)
            brow5_sb = res.tile([1, F5], BF16)
            hcrep_sb = res.tile([P, FM], F32)
            b6rep_sb = res.tile([P, FO], F32)
            dinv2_sb = res.tile([P, T], F32)
            pcol_sb = res.tile([P, 1], F32)
            for name, t_sb in [("gidx", gidx_sb), ("slotv", slot_sb),
                               ("normv", norm_sb), ("iotab", iota_sb),
                               ("identb", ident_sb), ("onesb", ones_sb),
                               ("brow14", brow14_sb), ("brow5", brow5_sb),
                               ("hcrep", hcrep_sb), ("b6rep", b6rep_sb),
                               ("dinv2", dinv2_sb), ("pcol", pcol_sb)]:
                nc.sync.dma_start(out=t_sb[:], in_=d[name][:, :])

            ohall = res.tile([P, T * C * P], BF16)
            for t in range(T):
                for c in range(C):
                    nc.vector.tensor_scalar(
                        out=ohall[:, (t * C + c) * P:(t * C + c + 1) * P],
                        in0=iota_sb[:],
                        scalar1=slot_sb[:, t * C + c:t * C + c + 1],
                        scalar2=norm_sb[:, t * C + c:t * C + c + 1],
                        op0=mybir.AluOpType.is_equal,
                        op1=mybir.AluOpType.mult,
                    )

            nidx_reg = nc.gpsimd.to_reg(C * P)

            def gather(sp, t, src_d, felem, dt, tag):
                """msg[p, c*felem:(c+1)*felem] = src[idx[c*128+p], :]"""
                m = sp.tile([P, C * felem], dt, tag=tag, name=f"{tag}_{t}")
                nc.gpsimd.dma_gather(
                    out_ap=m[:].rearrange("p (c f) -> p c f", c=C),
                    in_ap=src_d[:, :],
                    idxs_ap=gidx_sb[:, t * IW:(t + 1) * IW],
                    num_idxs=C * P,
                    num_idxs_reg=nidx_reg,
                    elem_size=felem,
                    single_packet=False,
                )
                return m

            def build_diag(sp, t, tag):
                """diag[p, j] = (j==p) / deg[t*128+p], bf16."""
                dg = sp.tile([P, P], BF16, tag=tag, name=f"{tag}_{t}")
                nc.vector.tensor_scalar(
                    out=dg[:],
                    in0=iota_sb[:],
                    scalar1=pcol_sb[:, 0:1],
                    scalar2=dinv2_sb[:, t:t + 1],
                    op0=mybir.AluOpType.is_equal,
                    op1=mybir.AluOpType.mult,
                )
                return dg

            def build_onehot(sp, t, tag):
                """oh[p, c*P+j] = (slot[p,c]==j) * norm[p,c], bf16."""
                oh = sp.tile([P, C * P], BF16, tag=tag, name=f"{tag}_{t}")
                for c in range(C):
                    nc.vector.tensor_scalar(
                        out=oh[:, c * P:(c + 1) * P],
                        in0=iota_sb[:],
                        scalar1=slot_sb[:, t * C + c:t * C + c + 1],
                        scalar2=norm_sb[:, t * C + c:t * C + c + 1],
                        op0=mybir.AluOpType.is_equal,
                        op1=mybir.AluOpType.mult,
                    )
                return oh

            # ---- layer 1 dense: h1 = verts @ W1[:3] + (img @ W1[3:]) ----
            with tc.tile_pool(name="l1", bufs=1) as l1p, \
                 tc.tile_pool(name="l1ps", bufs=2, space="PSUM") as l1ps, \
                 tc.tile_pool(name="l1sb", bufs=3) as l1sb:
                xT1_sb = l1p.tile([3, NP], BF16)
                nc.sync.dma_start(out=xT1_sb[:], in_=d["xT1"][:, :])
                W1v_sb = l1p.tile([3, FM], BF16)
                nc.sync.dma_start(out=W1v_sb[:], in_=d["W1v"][:, :])
                for n in range(T):
                    ph = l1ps.tile([P, FM], F32, tag="ph")
                    nc.tensor.matmul(out=ph[:], lhsT=xT1_sb[:, n * P:(n + 1) * P],
                                     rhs=W1v_sb[:], start=True, stop=True)
                    hs = l1sb.tile([P, FM], BF16, tag="hs")
                    nc.vector.tensor_add(out=hs[:], in0=ph[:], in1=hcrep_sb[:])
                    nc.sync.dma_start(out=h512[0][n * P:(n + 1) * P, :], in_=hs[:])

            # ---- phases i=1..4: scatter(i) + dense(i+1) per dst tile ----
            for i in (1, 2, 3, 4):
                relu = i in (2, 4)
                h_src = h512[i - 1]
                F_out = FM if i < 4 else F5
                h_dst = h512[i] if i < 4 else h5_d
                W_d = d[f"W{i + 1}"]
                with tc.tile_pool(name=f"ph{i}", bufs=SBUFS) as sp, \
                     tc.tile_pool(name=f"ph{i}w", bufs=1) as wp, \
                     tc.tile_pool(name=f"ph{i}ps", bufs=PSBUFS, space="PSUM") as pp, \
                     tc.tile_pool(name=f"ph{i}pt", bufs=2, space="PSUM") as pt, \
                     tc.tile_pool(name=f"ph{i}pd", bufs=2, space="PSUM") as pd:
                    W_sb = [wp.tile([P, F_out], BF16, tag=f"w{k}", name=f"w{i}_{k}")
                            for k in range(KM)]
                    for k in range(KM):
                        nc.sync.dma_start(out=W_sb[k][:], in_=W_d[k * P:(k + 1) * P, :])
                    for t in range(T):
                        msg = gather(sp, t, h_src, FM, BF16, "msg")
                        hre = sp.tile([P, FM], BF16, tag="hre", name=f"hre{i}_{t}")
                        nc.sync.dma_start(out=hre[:],
                                          in_=h_src[t * P:(t + 1) * P, :])
                        dg = build_diag(sp, t, "dg")
                        pa = pp.tile([P, FM], F32, tag="pa", name=f"pa{i}_{t}")
                        if HASB:
                            nc.tensor.matmul(
                                out=pa[:], lhsT=ones_sb[:],
                                rhs=brow14_sb[:, (i - 1) * FM:i * FM],
                                start=True, stop=False)
                        nc.tensor.matmul(
                            out=pa[:], lhsT=dg[:], rhs=hre[:],
                            start=not HASB, stop=False)
                        for c in range(C):
                            nc.tensor.matmul(
                                out=pa[:],
                                lhsT=ohall[:, (t * C + c) * P:(t * C + c + 1) * P],
                                rhs=msg[:, c * FM:(c + 1) * FM],
                                start=False, stop=(c == C - 1))
                        node = sp.tile([P, FM], BF16, tag="node", name=f"nd{i}_{t}")
                        nc.scalar.activation(out=node[:], in_=pa[:],
                                             func=Relu if relu else Ident, bias=0.0)
                        ptr = pt.tile([P, FM], BF16, tag="ptr", name=f"pt{i}_{t}")
                        stage = sp.tile([P, FM], BF16, tag="stage", name=f"st{i}_{t}")
                        for fo in range(KM):
                            nc.tensor.matmul(
                                out=ptr[:, fo * P:(fo + 1) * P],
                                lhsT=node[:, fo * P:(fo + 1) * P],
                                rhs=ident_sb[:], is_transpose=True,
                                start=True, stop=True)
                            nc.scalar.activation(
                                out=stage[:, fo * P:(fo + 1) * P],
                                in_=ptr[:, fo * P:(fo + 1) * P],
                                func=Ident, bias=0.0)
                        ph = pd.tile([P, F_out], F32, tag="ph", name=f"pd{i}_{t}")
                        for k in range(KM):
                            nc.tensor.matmul(out=ph[:], lhsT=stage[:, k * P:(k + 1) * P],
                                             rhs=W_sb[k][:], start=(k == 0),
                                             stop=(k == KM - 1))
                        if i < 4:
                            hs = sp.tile([P, F_out], BF16, tag="hs",
                                         name=f"hs{i}_{t}")
                            nc.vector.tensor_copy(out=hs[:], in_=ph[:])
                            nc.sync.dma_start(out=h_dst[t * P:(t + 1) * P, :],
                                              in_=hs[:])
                        else:
                            hs = sp.tile([P, 2 * F_out], BF16, tag="hs",
                                         name=f"hs{i}_{t}")
                            nc.vector.tensor_copy(out=hs[:, :F_out], in_=ph[:])
                            nc.vector.memset(hs[:, F_out:], 0.0)
                            nc.sync.dma_start(out=h_dst[t * P:(t + 1) * P, :],
                                              in_=hs[:])

            # ---- layer 5 scatter: x6 = A @ h5 + b5 ----
            with tc.tile_pool(name="s5", bufs=SBUFS) as sp5, \
                 tc.tile_pool(name="s5ps", bufs=PSBUFS, space="PSUM") as pp5:
                for t in range(T):
                    msg5b = gather(sp5, t, h5_d, 2 * F5, BF16, "m5")
                    hre5 = sp5.tile([P, F5], BF16, tag="hre5", name=f"hr5_{t}")
                    nc.sync.dma_start(out=hre5[:],
                                      in_=h5_d[t * P:(t + 1) * P, :F5])
                    dg5 = build_diag(sp5, t, "dg5")
                    pa5 = pp5.tile([P, F5], F32, tag="pa5", name=f"pa5_{t}")
                    if HASB:
                        nc.tensor.matmul(out=pa5[:], lhsT=ones_sb[:], rhs=brow5_sb[:],
                                         start=True, stop=False)
                    nc.tensor.matmul(out=pa5[:], lhsT=dg5[:], rhs=hre5[:],
                                     start=not HASB, stop=False)
                    for c in range(C):
                        nc.tensor.matmul(
                            out=pa5[:],
                            lhsT=ohall[:, (t * C + c) * P:(t * C + c + 1) * P],
                            rhs=msg5b[:, 2 * c * F5:2 * c * F5 + F5],
                            start=False, stop=(c == C - 1))
                    x6t = sp5.tile([P, 2 * F5], BF16, tag="x6t", name=f"x6_{t}")
                    nc.scalar.activation(out=x6t[:, :F5], in_=pa5[:], func=Ident,
                                         bias=0.0)
                    nc.vector.memset(x6t[:, F5:], 0.0)
                    nc.sync.dma_start(out=x6_d[t * P:(t + 1) * P, :], in_=x6t[:])

            # ---- layer 6: feature-major scatter, then 64->3 dense ----
            with tc.tile_pool(name="s6", bufs=SBUFS) as sp6, \
                 tc.tile_pool(name="s6w", bufs=1) as wp6, \
                 tc.tile_pool(name="s6ps", bufs=2, space="PSUM") as pp6, \
                 tc.tile_pool(name="s6pd", bufs=2, space="PSUM") as pd6:
                W6_sb = wp6.tile([F5, FO], F32)
                nc.sync.dma_start(out=W6_sb[:], in_=d["W6"][:, :])
                for t in range(T):
                    msg6b = gather(sp6, t, x6_d, 2 * F5, BF16, "m6")
                    hre6 = sp6.tile([P, F5], BF16, tag="hre6", name=f"hr6_{t}")
                    nc.sync.dma_start(out=hre6[:],
                                      in_=x6_d[t * P:(t + 1) * P, :F5])
                    dg6 = build_diag(sp6, t, "dg6")
                    pg = pp6.tile([F5, P], F32, tag="pg", name=f"pg_{t}")
                    nc.tensor.matmul(out=pg[:], lhsT=hre6[:], rhs=dg6[:],
                                     start=True, stop=False)
                    for c in range(C):
                        nc.tensor.matmul(
                            out=pg[:], lhsT=msg6b[:, 2 * c * F5:2 * c * F5 + F5],
                            rhs=ohall[:, (t * C + c) * P:(t * C + c + 1) * P],
                            start=False, stop=(c == C - 1))
                    gst = sp6.tile([F5, P], F32, tag="gst", name=f"g_{t}")
                    nc.scalar.activation(out=gst[:], in_=pg[:], func=Ident, bias=0.0)
                    pf = pd6.tile([P, FO], F32, tag="pf", name=f"pf_{t}")
                    nc.tensor.matmul(out=pf[:], lhsT=gst[:], rhs=W6_sb[:],
                                     start=True, stop=True)
                    os_ = sp6.tile([P, FO], F32, tag="os", name=f"o_{t}")
                    nc.vector.tensor_add(out=os_[:], in0=pf[:], in1=b6rep_sb[:])
                    nc.sync.dma_start(out=out_d[t * P:(t + 1) * P, :], in_=os_[:])

    mybir.codegen_inst_isa_subclasses(nc)
    _split_multiwaits(nc)
    return nc


def _bf16(a):
    import ml_dtypes
    return np.ascontiguousarray(np.asarray(a, np.float32).astype(ml_dtypes.bfloat16))


def _prepare(batch_vertices, img_features, edge_indices,
             W1, b1, W2, b2, W3, b3, W4, b4, W5, b5, W6, b6):
    B, N, _ = batch_vertices.shape
    FM = W1.shape[1]
    F5 = W5.shape[1]
    FO = W6.shape[1]

    ei = np.asarray(edge_indices).astype(np.int64)
    g = _pack_graph(ei[0], ei[1], N)
    NP, T, C, perm = g["NP"], g["T"], g["C"], g["perm"]

    hc = img_features.astype(np.float32) @ W1[3:].astype(np.float32)

    valid = perm >= 0
    vperm = np.zeros((B, NP, 3), np.float32)
    vperm[:, valid, :] = batch_vertices[:, perm[valid], :]

    iota = np.tile(np.arange(P, dtype=np.float32), (P, 1))
    ident = np.eye(P, dtype=np.float32)

    common = {
        "W1v": _bf16(W1[:3]),
        "W2": _bf16(W2),
        "W3": _bf16(W3),
        "W4": _bf16(W4),
        "W5": _bf16(W5),
        "W6": np.ascontiguousarray(W6.astype(np.float32)),
        "b6rep": np.tile(b6.astype(np.float32), (P, 1)),
        "gidx": g["gidx"],
        "slotv": np.ascontiguousarray(g["slot"]),
        "normv": np.ascontiguousarray(g["norm"]),
        "iotab": _bf16(iota),
        "identb": _bf16(ident),
        "onesb": _bf16(np.ones((1, P), np.float32)),
        "brow14": _bf16(np.concatenate([b1, b2, b3, b4])[None, :]),
        "brow5": _bf16(b5[None, :]),
        "dinv2": g["dinv2"],
        "pcol": np.arange(P, dtype=np.float32)[:, None].copy(),
    }
    in_maps = []
    for b in range(B):
        m = dict(common)
        m["xT1"] = _bf16(vperm[b].T)
        m["hcrep"] = np.tile(hc[b], (P, 1)).astype(np.float32)
        in_maps.append(m)
    meta = dict(NP=NP, T=T, C=C, perm=perm, valid=valid, B=B, N=N,
                FM=FM, F5=F5, FO=FO)
    return in_maps, meta


_BUILD_CACHE = {}


def run(inputs, trace=False):
    in_maps, meta = _prepare(**inputs)
    hasb = any(np.any(np.asarray(inputs[f"b{i}"])) for i in range(1, 6))
    os.environ["KBASS_HASBIAS"] = "1" if hasb else "0"
    key = (meta["NP"], meta["C"], meta["FM"], meta["F5"], meta["FO"], hasb)
    if key not in _BUILD_CACHE:
        t0 = time.time()
        _BUILD_CACHE[key] = _build_nc(meta["NP"], meta["T"], meta["C"],
                                      meta["FM"], meta["F5"], meta["FO"])
        print(f"[kernel] built bass program in {time.time()-t0:.1f}s", file=sys.stderr)
    nc = _BUILD_CACHE[key]
    B = meta["B"]
    res = run_bass_kernel_spmd(nc, in_maps, core_ids=list(range(B)), trace=trace)
    perm, valid, N = meta["perm"], meta["valid"], meta["N"]
    out = np.empty((B, N, meta["FO"]), np.float32)
    for b in range(B):
        dev = res.results[b]["out"]
        out[b, perm[valid], :] = dev[valid, :]
    return out, res


def kernel(**inputs) -> np.ndarray:
    out, _ = run(inputs)
    return out
